# revision 9
# baseline (speedup 1.0000x reference)
"""Causal multi-head self-attention with RoPE on 8 Trainium2 NeuronCores.

Sharding: heads are split across the 8 cores (2 heads each). Each core
computes Q/K/V projections for its heads over the full sequence, causal
flash-style attention, and a partial output projection against its slice
of Wo (row-parallel). The host sums the 8 partial outputs (the unshard
step of a row-parallel linear).

All matmuls run in bf16 with fp32 PSUM accumulation. Softmax runs without
max-subtraction (scores are ~N(0,1) by construction, exp is safe in fp32)
using a ones-column folded into the V operand to produce denominators.
"""
import numpy as np
import ml_dtypes

import concourse.bass as bass
import concourse.mybir as mybir
import concourse.tile as tile
from concourse.vector_clock import ScopedClock
from concourse.tile_rust import add_dep_helper
from concourse.bass_utils import run_bass_kernel_spmd

F32 = mybir.dt.float32
BF16 = mybir.dt.bfloat16
AF = mybir.ActivationFunctionType

N_CORES = 8
B, S, D = 2, 2048, 1024
H, DK = 16, 64
HL = H // N_CORES          # heads per core = 2
BS = B * S                 # 4096 flattened sequence
NT = S // 128              # 16 sk tiles per batch
NQ = S // 512              # 4 sq blocks per batch
THETA = 10000.0
SCALE = 1.0 / float(np.sqrt(DK))


def split_multi_waits(nc):
    """This walrus build rejects >1 sync wait per instruction: move extra
    waits onto single-wait NOPs inserted just before, on the same engine."""
    for fn in nc.m.functions:
        for bb in fn.blocks:
            insts = bb.instructions
            i = 0
            while i < len(insts):
                inst = insts[i]
                si = inst.sync_info
                if si is not None and len(si.on_wait) > 1:
                    waits = list(si.on_wait)
                    for w in waits[:-1]:
                        nop = mybir.InstNoOp(
                            name=f"I-{nc.next_id()}",
                            engine=inst.engine,
                            ins=[],
                            outs=[],
                            sync_info=mybir.SyncInfo(on_wait=[w], on_update=[]),
                        )
                        insts.insert(i, nop)
                        i += 1
                    del si.on_wait[:-1]
                i += 1


class TC(tile.TileContext):
    """TileContext with single-wait splitting for this walrus build."""

    def __exit__(self, *args):
        r = super().__exit__(*args)
        split_multi_waits(self.nc)
        return r

    def _drain_and_barrier(self, tick_clock, wait_clock):
        probe = self.nc.sync.nop()
        wait_clock.add_sem_waits(probe.ins, ScopedClock({None: tick_clock.global_clock}))
        waits = list(probe.ins.sync_info.on_wait)
        del probe.ins.sync_info.on_wait[0:]
        for w in waits:
            n = self.nc.sync.nop()
            n.ins.sync_info = mybir.SyncInfo(on_wait=[w], on_update=[])
        self.nc.sync.drain()
        self.nc.all_engine_barrier()
        assert self.sems is not None
        popped = self.nc._tile_sem_poison_stack.pop()
        assert popped is self._sem_poison
        self.nc.clear_and_free_semaphores(list(self.sems.allocated().values()))
        self.nc.all_engine_barrier()


def seg_off(t):
    """Column offset of sk-tile t's probsT segment; tile t covers sq cols
    [qb(t), S) with qb = 512*(t//4)."""
    off = 0
    for u in range(t):
        off += S - 512 * (u // 4)
    return off


SEG = [seg_off(t) for t in range(NT)]
SEG_TOTAL = seg_off(NT)  # 20480
VA_STRIDE = 80  # 64 v cols + 1 ones col + 15 pad (160B alignment for DMA transpose)


def build_nc(debug=False):
    nc = bass.Bass("TRN2", target_bir_lowering=False, debug=False, num_devices=N_CORES)

    xT_d = nc.dram_tensor("xT", [D, BS], BF16, kind="ExternalInput")
    wq_d = nc.dram_tensor("wqT", [D, 128], BF16, kind="ExternalInput")
    wk_d = nc.dram_tensor("wkT", [D, 128], BF16, kind="ExternalInput")
    wv_d = nc.dram_tensor("wvT", [D, 128], BF16, kind="ExternalInput")
    wo_d = nc.dram_tensor("woT", [128, D], BF16, kind="ExternalInput")
    cc_d = nc.dram_tensor("ccn", [128, BS], F32, kind="ExternalInput")
    ss_d = nc.dram_tensor("ssn", [128, BS], F32, kind="ExternalInput")
    tri_d = nc.dram_tensor("tri", [128, 128], BF16, kind="ExternalInput")
    ones_d = nc.dram_tensor("ones", [128, 64], BF16, kind="ExternalInput")
    zeros_d = nc.dram_tensor("zeros", [128, 384], BF16, kind="ExternalInput")
    y_d = nc.dram_tensor("y_part", [BS, D], F32, kind="ExternalOutput")
    if debug:
        dbg_qrot = nc.dram_tensor("dbg_qrot", [128, BS], BF16, kind="ExternalOutput")
        dbg_krot = nc.dram_tensor("dbg_krot", [128, BS], BF16, kind="ExternalOutput")
        dbg_va = nc.dram_tensor("dbg_va", [128, B * HL * NT * VA_STRIDE], BF16, kind="ExternalOutput")
        dbg_attnT = nc.dram_tensor("dbg_attnT", [128, BS], BF16, kind="ExternalOutput")
        dbg_probs = nc.dram_tensor("dbg_probs", [128, SEG_TOTAL], BF16, kind="ExternalOutput")
        dbg_dn = nc.dram_tensor("dbg_dn", [1, 2048], F32, kind="ExternalOutput")

    with TC(nc) as tc:
        with (
            tc.tile_pool(name="persist", bufs=1) as pp,
        ):
            wq_sb = pp.tile([128, 1024], BF16)
            wk_sb = pp.tile([128, 1024], BF16)
            wv_sb = pp.tile([128, 1024], BF16)
            wo_sb = pp.tile([128, 1024], BF16)
            tri_sb = pp.tile([128, 128], BF16)
            ones_sb = pp.tile([128, 64], BF16)
            zeros_sb = pp.tile([128, 384], BF16)
            qrotT = pp.tile([128, BS], BF16)
            krotT = pp.tile([128, BS], BF16)
            va = pp.tile([128, B * HL * NT * VA_STRIDE], BF16)  # [b][h][t] -> [128, 65] (+pad)
            attnT = pp.tile([128, BS], BF16)

            for k in range(8):
                nc.sync.dma_start(wq_sb[:, 128 * k:128 * (k + 1)], wq_d[128 * k:128 * (k + 1), :])
                nc.sync.dma_start(wk_sb[:, 128 * k:128 * (k + 1)], wk_d[128 * k:128 * (k + 1), :])
                nc.sync.dma_start(wv_sb[:, 128 * k:128 * (k + 1)], wv_d[128 * k:128 * (k + 1), :])
            nc.sync.dma_start(wo_sb[:], wo_d[:])
            nc.sync.dma_start(tri_sb[:], tri_d[:])
            nc.sync.dma_start(ones_sb[:], ones_d[:])
            nc.sync.dma_start(zeros_sb[:], zeros_d[:])
            # ones column of every v tile (plain single-column DMAs)
            for t in range(B * HL * NT):
                nc.sync.dma_start(
                    va[:, VA_STRIDE * t + 64:VA_STRIDE * t + 65], ones_sb[:, 0:1])

            # ---------- Phase 1: projections + rope + v transpose (per batch) ----------
            # DmaTransposeAnt is not dep-tracked by Tile: all transposes get
            # explicit add_dep_helper edges (RAW in), and consumers depend on
            # per-group joiner nops (RAW out). Transpose src/dst tiles are
            # never pooled/reused, so no WAR/WAW hazards remain.
            rot_join = {}
            va_join = {}
            with (
                tc.tile_pool(name="ph1sb", bufs=1) as p1,
                tc.tile_pool(name="ph1ps", bufs=4, space="PSUM") as ps1,
            ):
                vT = p1.tile([128, BS], BF16)
                for bh in range(B):
                    c0 = S * bh  # column base of this batch in BS-wide tensors
                    xt = p1.tile([128, 8 * S], BF16, name=f"xt_{bh}", tag="xt", bufs=2)
                    for k in range(8):
                        nc.sync.dma_start(
                            xt[:, S * k:S * (k + 1)],
                            xT_d[128 * k:128 * (k + 1), c0:c0 + S])
                    cc_sb = p1.tile([128, S], F32, name=f"cc_{bh}", tag="cc", bufs=1)
                    ss_sb = p1.tile([128, S], F32, name=f"ss_{bh}", tag="ss", bufs=1)
                    nc.sync.dma_start(cc_sb[:], cc_d[:, c0:c0 + S])
                    nc.sync.dma_start(ss_sb[:], ss_d[:, c0:c0 + S])

                    rot_trs = []
                    for name, w_sb, rotT in (("q", wq_sb, qrotT), ("k", wk_sb, krotT)):
                        projT = p1.tile([128, S], BF16, name=f"projT_{name}{bh}")
                        evacs = []
                        for nb in range(4):
                            pps = ps1.tile([128, 512], F32, name=f"pps_{name}{bh}{nb}", tag="pps")
                            for k in range(8):
                                nc.tensor.matmul(
                                    pps[:],
                                    w_sb[:, 128 * k:128 * (k + 1)],
                                    xt[:, S * k + 512 * nb: S * k + 512 * (nb + 1)],
                                    start=(k == 0), stop=(k == 7),
                                )
                            evacs.append(nc.scalar.copy(projT[:, 512 * nb:512 * (nb + 1)], pps[:]))
                        # transpose to natural layout [seq, dims] per 128-col tile
                        nat = p1.tile([128, S], BF16, name=f"nat_{name}{bh}")
                        nat_trs = []
                        for t in range(NT):
                            tr = nc.sync.dma_start(
                                nat[:, 128 * t:128 * (t + 1)],
                                projT[:, 128 * t:128 * (t + 1)],
                                transpose=True,
                            )
                            add_dep_helper(tr.ins, evacs[t // 4].ins, reason="xpose RAW in")
                            nat_trs.append(tr)
                        njoin = nc.sync.nop()
                        for tr in nat_trs:
                            add_dep_helper(njoin.ins, tr.ins, reason="nat join")
                        # rope: rot = nat*cos + pairswap(nat)*sgn_sin
                        tmp1 = p1.tile([128, S], F32, name=f"tmp1_{name}{bh}", tag="tmp1", bufs=1)
                        tmp2 = p1.tile([128, S], F32, name=f"tmp2_{name}{bh}", tag="tmp2", bufs=1)
                        m1 = nc.vector.tensor_tensor(tmp1[:], nat[:], cc_sb[:], op=mybir.AluOpType.mult)
                        add_dep_helper(m1.ins, njoin.ins, reason="rope after nat")
                        nat_sw = nat[:].rearrange("p (n two) -> p n two", two=2)[:, :, ::-1]
                        m2 = nc.vector.tensor_tensor(
                            tmp2[:].rearrange("p (n two) -> p n two", two=2),
                            nat_sw,
                            ss_sb[:].rearrange("p (n two) -> p n two", two=2),
                            op=mybir.AluOpType.mult,
                        )
                        add_dep_helper(m2.ins, njoin.ins, reason="rope after nat")
                        rot = p1.tile([128, S], BF16, name=f"rot_{name}{bh}")
                        radd = nc.vector.tensor_add(rot[:], tmp1[:], tmp2[:])
                        for t in range(NT):
                            tr = nc.sync.dma_start(
                                rotT[:, c0 + 128 * t:c0 + 128 * (t + 1)],
                                rot[:, 128 * t:128 * (t + 1)],
                                transpose=True,
                            )
                            add_dep_helper(tr.ins, radd.ins, reason="xpose RAW in")
                            rot_trs.append(tr)
                    rj = nc.sync.nop()
                    for tr in rot_trs:
                        add_dep_helper(rj.ins, tr.ins, reason="rot join")
                    rot_join[bh] = rj

                    # v: project into vT (transposed layout), no rope
                    vevacs = []
                    for nb in range(4):
                        pps = ps1.tile([128, 512], F32, name=f"pps_v{bh}{nb}", tag="pps")
                        for k in range(8):
                            nc.tensor.matmul(
                                pps[:],
                                wv_sb[:, 128 * k:128 * (k + 1)],
                                xt[:, S * k + 512 * nb: S * k + 512 * (nb + 1)],
                                start=(k == 0), stop=(k == 7),
                            )
                        vevacs.append(nc.scalar.copy(vT[:, c0 + 512 * nb:c0 + 512 * (nb + 1)], pps[:]))
                    va_trs = []
                    for h in range(HL):
                        for t in range(NT):
                            col = ((bh * HL + h) * NT + t) * VA_STRIDE
                            tr = nc.sync.dma_start(
                                va[:, col:col + 64],
                                vT[64 * h:64 * (h + 1), c0 + 128 * t: c0 + 128 * (t + 1)],
                                transpose=True,
                            )
                            add_dep_helper(tr.ins, vevacs[t // 4].ins, reason="xpose RAW in")
                            va_trs.append(tr)
                    vj = nc.sync.nop()
                    for tr in va_trs:
                        add_dep_helper(vj.ins, tr.ins, reason="va join")
                    va_join[bh] = vj

            # ---------------- Phase 2: attention per (b, h) --------------
            with (
                tc.tile_pool(name="ph2sb", bufs=1) as p2,
                tc.tile_pool(name="ph2ps", bufs=1, space="PSUM") as ps2,
            ):
                for b in range(B):
                    for h in range(HL):
                        hq = 64 * h
                        probs = p2.tile([128, SEG_TOTAL], BF16, name=f"probs_{b}{h}", tag="probs", bufs=2)
                        # scores + exp
                        for t in range(NT):
                            qb = 512 * (t // 4)
                            o = 128 * (t % 4)
                            width = S - qb
                            if o > 0:
                                nc.sync.dma_start(
                                    probs[:, SEG[t]:SEG[t] + o], zeros_sb[:, :o])
                            for cst in range(0, width, 1024):
                                cw = min(1024, width - cst)
                                sps = ps2.tile([128, 1024], F32, name=f"sps_{b}{h}{t}{cst}", tag="sps", bufs=2)
                                for j in range(0, cw, 512):
                                    smm = nc.tensor.matmul(
                                        sps[:, j:j + 512],
                                        krotT[hq:hq + 64, S * b + 128 * t: S * b + 128 * (t + 1)],
                                        qrotT[hq:hq + 64, S * b + qb + cst + j: S * b + qb + cst + j + 512],
                                        start=True, stop=True,
                                    )
                                    add_dep_helper(smm.ins, rot_join[b].ins, reason="scores after rot")
                                skip = o if cst == 0 else 0
                                nc.scalar.activation(
                                    probs[:, SEG[t] + cst + skip: SEG[t] + cst + cw],
                                    sps[:, skip:cw],
                                    AF.Exp, scale=SCALE,
                                )
                            # triangular boundary mask
                            nc.vector.tensor_tensor(
                                probs[:, SEG[t] + o: SEG[t] + o + 128],
                                probs[:, SEG[t] + o: SEG[t] + o + 128],
                                tri_sb[:],
                                op=mybir.AluOpType.mult,
                            )
                        if debug and b == 0 and h == 0:
                            nc.sync.dma_start(dbg_probs[:], probs[:])
                        # attnV with folded denominators
                        av_tiles = []
                        for q in range(NQ):
                            av = ps2.tile([65, 512], F32, name=f"av_{b}{h}{q}", tag="av", bufs=4)
                            tmax = 4 * (q + 1)
                            for t in range(tmax):
                                col = ((b * HL + h) * NT + t) * VA_STRIDE
                                avmm = nc.tensor.matmul(
                                    av[:],
                                    va[:, col:col + 65],
                                    probs[:, SEG[t] + 512 * q - 512 * (t // 4): SEG[t] + 512 * (q + 1) - 512 * (t // 4)],
                                    start=(t == 0), stop=(t == tmax - 1),
                                )
                                add_dep_helper(avmm.ins, va_join[b].ins, reason="attnV after va")
                            av_tiles.append(av)
                        # normalization: recip via exp(-ln), broadcast via K=1 matmul
                        dn = p2.tile([1, 2048], F32, name=f"dn_{b}{h}", tag="dn", bufs=2)
                        rc = p2.tile([1, 2048], BF16, name=f"rc_{b}{h}", tag="rc", bufs=2)
                        for q in range(NQ):
                            nc.scalar.activation(dn[0:1, 512 * q:512 * (q + 1)], av_tiles[q][64:65, :], AF.Ln)
                        nc.scalar.activation(rc[:], dn[:], AF.Exp, scale=-1.0)
                        if debug and b == 0 and h == 0:
                            nc.sync.dma_start(dbg_dn[:], dn[:])
                        for q in range(NQ):
                            bc_ps = ps2.tile([64, 512], F32, name=f"bc_{b}{h}{q}", tag="sps", bufs=2)
                            nc.tensor.matmul(
                                bc_ps[:],
                                ones_sb[0:1, :],
                                rc[0:1, 512 * q:512 * (q + 1)],
                                start=True, stop=True,
                            )
                            bc_sb = p2.tile([64, 512], BF16, name=f"bcs_{b}{h}{q}", tag="bcs", bufs=2)
                            nc.scalar.copy(bc_sb[:], bc_ps[:])
                            nc.vector.tensor_tensor(
                                attnT[hq:hq + 64, S * b + 512 * q: S * b + 512 * (q + 1)],
                                av_tiles[q][0:64, :],
                                bc_sb[:],
                                op=mybir.AluOpType.mult,
                            )

            if debug:
                nc.sync.dma_start(dbg_attnT[:], attnT[:])
            # ---------------- Phase 3: partial output projection --------------
            with (
                tc.tile_pool(name="ph3sb", bufs=4) as p3,
                tc.tile_pool(name="ph3ps", bufs=4, space="PSUM") as ps3,
            ):
                for m in range(BS // 128):
                    for e in range(2):
                        yps = ps3.tile([128, 512], F32, name=f"yps_{m}{e}", tag="yps")
                        nc.tensor.matmul(
                            yps[:],
                            attnT[:, 128 * m:128 * (m + 1)],
                            wo_sb[:, 512 * e:512 * (e + 1)],
                            start=True, stop=True,
                        )
                        ysb = p3.tile([128, 512], F32, name=f"ysb_{m}{e}", tag="ysb")
                        if (m + e) % 2 == 0:
                            nc.scalar.copy(ysb[:], yps[:])
                        else:
                            nc.vector.tensor_copy(ysb[:], yps[:])
                        nc.sync.dma_start(
                            y_d[128 * m:128 * (m + 1), 512 * e:512 * (e + 1)], ysb[:])
    return nc


_CACHED = {}


def _get_nc():
    if "nc" not in _CACHED:
        _CACHED["nc"] = build_nc()
    return _CACHED["nc"]


def _host_prep(x, Wq, Wk, Wv, Wo, token_positions):
    bf = ml_dtypes.bfloat16
    xT = np.ascontiguousarray(x.reshape(BS, D).T.astype(bf))

    # natural-layout rope tables [128 seq-in-tile, 32 tiles * 128 dims]
    pos = token_positions.astype(np.float32).reshape(BS)
    freq = (1.0 / (THETA ** (np.arange(0, DK, 2, dtype=np.float32) / DK))).astype(np.float32)
    ang_pair = pos[:, None] * freq[None, :]                        # [BS, 32] fp32
    cosv = np.cos(ang_pair)
    sinv = np.sin(ang_pair)
    d_idx = np.arange(128)
    pair = (d_idx % 64) // 2
    sgn = np.where(d_idx % 2 == 0, -1.0, 1.0).astype(np.float32)
    cc_full = cosv[:, pair]                                        # [BS, 128]
    ss_full = sinv[:, pair] * sgn[None, :]
    ccn = np.ascontiguousarray(cc_full.reshape(32, 128, 128).transpose(1, 0, 2).reshape(128, BS))
    ssn = np.ascontiguousarray(ss_full.reshape(32, 128, 128).transpose(1, 0, 2).reshape(128, BS))

    tri = np.ascontiguousarray(np.triu(np.ones((128, 128), np.float32)).astype(bf))
    ones = np.ones((128, 64), np.float32).astype(bf)
    zeros = np.zeros((128, 384), np.float32).astype(bf)

    in_maps = []
    for c in range(N_CORES):
        r0 = c * HL * DK
        in_maps.append({
            "xT": xT,
            "wqT": np.ascontiguousarray(Wq[r0:r0 + 128, :].T.astype(bf)),
            "wkT": np.ascontiguousarray(Wk[r0:r0 + 128, :].T.astype(bf)),
            "wvT": np.ascontiguousarray(Wv[r0:r0 + 128, :].T.astype(bf)),
            "woT": np.ascontiguousarray(Wo[:, r0:r0 + 128].T.astype(bf)),
            "ccn": ccn,
            "ssn": ssn,
            "tri": tri,
            "ones": ones,
            "zeros": zeros,
        })
    return in_maps


def kernel(x, Wq, Wk, Wv, Wo, token_positions, trace=False):
    in_maps = _host_prep(np.asarray(x), np.asarray(Wq), np.asarray(Wk),
                         np.asarray(Wv), np.asarray(Wo), np.asarray(token_positions))
    nc = _get_nc()
    res = run_bass_kernel_spmd(nc, in_maps, list(range(N_CORES)), trace=trace)
    y = np.zeros((BS, D), np.float64)
    for c in range(N_CORES):
        y += res.results[c]["y_part"].astype(np.float64)
    out = y.reshape(B, S, D).astype(np.float32)
    kernel.last_exec_time_ns = res.exec_time_ns
    return out
